# revision 23
# baseline (speedup 1.0000x reference)
"""Trainium2 Bass kernel for gated-attention pooling (nn_AttentionGated).

Computation (reference):
    h = relu(x[0] @ W_feat.T + b_feat)        # [N, 768]
    a = relu(h @ W_a.T)                        # [N, 128]
    b = sigmoid(h @ W_b.T)                     # [N, 128]
    logits = (a*b) @ W_c.T                     # [N] -> softmax over N
    out = softmax(logits) @ h                  # [1, 768]

Strategy: shard N=50000 rows over 8 cores (6250 each, 49 tiles of 128 rows).
Each core accumulates P = sum_n w_n h_n and Z = sum_n w_n (w = exp(logit))
in persistent PSUM banks; the host merges partial (P, Z): out = sum P / sum Z.
No on-device collective.

Optimizations over the first working version (165 us):
  * b_feat is folded into x on the host (x' = x + b @ inv(W_feat^T), exact
    in exact arithmetic), killing the two per-tile bias matmuls (768 PE
    columns/tile, ~19 us of PE time).
  * sigmoid computed as (1 + tanh(x/2))/2 so every ACT function used (Relu,
    Tanh, Exp, Copy) lives in the one `exp_and_others` table set -- the
    Sigmoid<->Exp table reloads (15 x 1.3us on the ACT critical path) vanish.
    The 1/2 is folded into W_c host-side; the +1 costs one DVE tensor_scalar.
  * h is evacuated once (single 768-wide ACT relu from a 2-bank PSUM tile)
    straight to fp8, laid out [128, 2, 784] with the pair axis = tile parity.
    That makes h directly usable as the DoubleRow moving operand of the P/Z
    rank-1 update, so one P/Z matmul pair covers TWO row tiles (769 columns
    per 2 tiles instead of per tile).
  * hT (needed as the stationary operand of the a/b GEMM, which contracts
    over the feature dim) comes from 6 PE transposes at fp8 into a single
    fp8 PSUM bank, evacuated by one DVE copy into DoubleRow-interleaved
    layout; the a/b GEMM then runs fp8 DoubleRow (3 matmuls of N=256 vs 6
    bf16 matmuls).
  * the 6 transposes split across two single-bank psum tiles evacuated
    eagerly (3 chunks at a time) via a uint16-bitcast DVE copy (2x mode);
    the gate multiplies ride the otherwise-idle GpSimd engine; exp stays
    batched (8 tiles) purely to amortize ACT instruction overhead.
  * the per-tile dependency chain h -> hT -> a/b -> logits is software-
    pipelined across iterations (transposes lag 1 tile and are interleaved
    between the h matmuls to spread LDWEIGHTS-port load, a/b GEMM lags 3
    tiles, P/Z 4) so no engine waits in-line for another; a/b GEMM output
    parks in the unused upper half-bank of the h PSUM tile to fit in 8
    PSUM banks.  NOTE a start=True matmul clears bank-level accumulation
    state, so the a/b group must open only after the h group's stop.
  * DMA order matters: every matmul waits on the serial sync-queue DMA
    counter, so constants are queued in first-use order (bias, x tile 0,
    W_feat, ...), halving kernel-start latency.

fp8 notes: W_feat/b_feat and W_a|W_b are scaled x16 on the host to dodge
e4m3 subnormals; the ACT evacuations rescale by 1/16 (and the tanh by 1/32,
absorbing the sigmoid's x/2). Quantization noise averages out across the
50k-row softmax pooling.
"""

import sys
import types

import numpy as np
import ml_dtypes

import concourse.bass as bass
import concourse.bacc as bacc
import concourse.mybir as mybir
from concourse import tile
from concourse.bass_utils import run_bass_kernel_spmd

BF16 = ml_dtypes.bfloat16
FP8 = ml_dtypes.float8_e4m3
W_SCALE = 16.0  # fp8 subnormal-range dodge for W_feat/b_feat/W_ab

N_CORES = 8
N = 50000
DIM = 768
D_ATT = 128
NS = N // N_CORES            # 6250 rows per core
T = 49                       # tiles of 128 rows (6272 padded)
LAST_VALID = NS - (T - 1) * 128  # 106 valid rows in the last tile
BATCH = 8                    # exp batching (amortize ACT overhead)
HPAD = 784                   # h row pitch (784 % 16 == 0 for DR moving AP)

_cached_nc = None
last_results = None  # BassKernelResults of the most recent run (for profiling)


def _build_nc():
    AF = mybir.ActivationFunctionType
    DR = mybir.MatmulPerfMode.DoubleRow
    dt = mybir.dt

    nc = bacc.Bacc("TRN2", target_bir_lowering=False, debug=False)

    xt_d = nc.dram_tensor("xt", [T, 128, 3, 2, 128], dt.float8e4, kind="ExternalInput").ap()
    wt_d = nc.dram_tensor("wt", [128, 3, 2, DIM], dt.float8e4, kind="ExternalInput").ap()
    wab_d = nc.dram_tensor("wab", [128, 3, 2, 2 * D_ATT], dt.float8e4, kind="ExternalInput").ap()
    wcbc_d = nc.dram_tensor("wcbc", [128, D_ATT], dt.bfloat16, kind="ExternalInput").ap()
    mask_d = nc.dram_tensor("mask", [128, 1], dt.bfloat16, kind="ExternalInput").ap()
    ident_d = nc.dram_tensor("ident", [128, 128], dt.float8e4, kind="ExternalInput").ap()
    out_d = nc.dram_tensor("out", [1, DIM + 1], dt.float32, kind="ExternalOutput").ap()

    with tile.TileContext(nc) as tc:
        with (
            tc.tile_pool(name="const", bufs=1) as constp,
            tc.tile_pool(name="xtp", bufs=8) as xtp,
            tc.tile_pool(name="hp", bufs=16) as hp,
            tc.tile_pool(name="Hp", bufs=4) as Hp,
            tc.tile_pool(name="ltp", bufs=2) as ltp,
            tc.tile_pool(name="smp", bufs=3) as smp,
            tc.tile_pool(name="psh", bufs=2, space="PSUM") as pshp,
            tc.tile_pool(name="psT", bufs=2, space="PSUM") as psTp,
            tc.tile_pool(name="psacc", bufs=1, space="PSUM") as paccp,
        ):
            # --- constants (loaded once) ---
            # two HWDGE queues run in parallel: the sync (SP) queue carries
            # only the x tiles (tile 0 arrives ~0.6us in), while the scalar
            # (Activation) queue streams the weights (W_feat split per
            # s-chunk so the first h matmuls start after ~200KB, not 590KB).
            # b_feat is folded into x on the host (x' = x + b @
            # inv(W_feat^T)), so there is no bias work on device.
            xt_of = {}
            xt_of[0] = xtp.tile([128, 3, 2, 128], dt.float8e4, tag="xt",
                                name="xt")
            nc.sync.dma_start(xt_of[0][:], xt_d[0])
            wt_sb = constp.tile([128, 3, 2, DIM], dt.float8e4)
            nc.scalar.dma_start(wt_sb[:, 0:1], wt_d[:, 0:1])
            nc.scalar.dma_start(wt_sb[:, 1:2], wt_d[:, 1:2])
            nc.scalar.dma_start(wt_sb[:, 2:3], wt_d[:, 2:3])
            wab_sb = constp.tile([128, 3, 2, 2 * D_ATT], dt.float8e4)
            nc.scalar.dma_start(wab_sb[:], wab_d[:])
            wcbc_sb = constp.tile([128, D_ATT], dt.bfloat16)
            nc.scalar.dma_start(wcbc_sb[:], wcbc_d[:])
            ident_sb = constp.tile([128, 128], dt.float8e4)
            nc.scalar.dma_start(ident_sb[:], ident_d[:])
            mask_sb = constp.tile([128, 1], dt.bfloat16)
            nc.scalar.dma_start(mask_sb[:], mask_d[:])

            # persistent PSUM accumulator: one 2-bank tile [P | Z]; each
            # matmul output stays within a single bank (0:512 / 512:769)
            ppz = paccp.tile([1, 769], dt.float32, tag="ppz")
            ppza = ppz[:, 0:512]
            ppzb = ppz[:, 512:769]

            # live tiles per pipeline stage
            ph_of = {}    # iter -> psum [128, 1024] (h in 0:768, a|b in 768:1024)
            hD_of = {}    # pair index -> sbuf fp8 [128, 2, HPAD]
            Ht_of = {}    # tile -> sbuf fp8 [128, 3, 2, 128] (DR-packed hT)
            lt_of = {}    # batch -> sbuf fp32 [128, BATCH]
            started = [False]

            def stage_h(t, transp_mm=None):
                """DMA x tile; h-GEMM with the previous tile's transposes
                interleaved between the matmuls, so the LDWEIGHTS port load
                is spread evenly and never bubbles the PE stream."""
                if t in xt_of:
                    xt = xt_of.pop(t)
                else:
                    xt = xtp.tile([128, 3, 2, 128], dt.float8e4, tag="xt")
                    nc.sync.dma_start(xt[:], xt_d[t])
                ph = ph_of[t]
                def tr(c):
                    if transp_mm is not None:
                        transp_mm(c)
                for s in range(3):
                    nc.tensor.matmul(ph[:, 0:512], xt[:, s], wt_sb[:, s, :, 0:512],
                                     start=(s == 0), stop=(s == 2),
                                     perf_mode=DR, skip_group_check=True)
                    nc.tensor.matmul(ph[:, 512:768], xt[:, s], wt_sb[:, s, :, 512:768],
                                     start=(s == 0), stop=(s == 2),
                                     perf_mode=DR, skip_group_check=True)
                    tr(2 * s)
                    tr(2 * s + 1)
                par = t & 1
                if par == 0:
                    hD = hp.tile([128, 2, HPAD], dt.float8e4, tag="hD",
                                 name="hD")
                    hD_of[t // 2] = hD
                else:
                    hD = hD_of[t // 2]
                if t == T - 1:
                    # lone tile of the final pair: zero the unused pair slot so
                    # the paired P/Z matmul sees 0*0 (its w slot is zeroed too).
                    nc.vector.memset(hD[:, 1, :], 0.0)
                nc.scalar.activation(hD[:, par, 0:768], ph[:, 0:768], AF.Relu,
                                     scale=1.0 / W_SCALE)
                # column 768 = softmax-denominator ones column (dead rows are
                # killed via their w, so plain ones everywhere is fine)
                nc.vector.memset(hD[:, par, 768:769], 1.0)

            def alloc_psT():
                """One psum bank: 768B of transposed-h (fp8, stride-2 slots)
                plus a 1KB corner holding a 256-col fp32 a|b GEMM output.
                Sharing the bank costs nothing: the transposes are singleton
                matmuls and the a|b group is emitted closed (stop) before the
                first transpose of the same bank."""
                return psTp.tile([128, 1792], dt.float8e4, tag="psT",
                                 name="psT")

            def make_transp(t, psTs):
                """Per-chunk fp8 PE transpose emitters for tile t (called
                interleaved from stage_h). The 6 transposes split across two
                single-bank psum tiles, each evacuated as soon as its 3
                chunks land, so the next tile's transposes only ever wait on
                a half-bank evacuation. Evacs copy the stride-2 fp8 psum
                bitcast to uint16 (2x DVE mode); the a|b stationary reads
                the fp8 values back with a stride-2 AP."""
                par = t & 1
                hD = hD_of[t // 2]
                Ht = Hp.tile([128, 768, 2], dt.float8e4, tag="Ht", name="Ht")
                Ht_of[t] = Ht
                def emit(c):
                    half = c // 3
                    psT = psTs[half][:, 0:768].rearrange("p (n q) -> p n q",
                                                         q=2)
                    nc.tensor.transpose(psT[:, bass.ts(c % 3, 128), 0:1],
                                        hD[:, par, bass.ts(c, 128)],
                                        ident_sb[:])
                    if c % 3 == 2:
                        dst = Ht[:, bass.ts(half, 384)]
                        nc.vector.tensor_copy(dst.bitcast(dt.uint16),
                                              psT.bitcast(dt.uint16))
                return emit

            def emit_ab(t, pab):
                """a|b GEMM into the psT-bank corner (decouples ph reuse
                from the gate chain)."""
                Ht = Ht_of.pop(t)
                HtS = Ht[:].rearrange("p (s j n) q -> p s j n q",
                                      s=3, j=2, n=128)
                for s in range(3):
                    nc.tensor.matmul(pab, HtS[:, s, :, :, 0:1], wab_sb[:, s],
                                     start=(s == 0), stop=(s == 2),
                                     perf_mode=DR, skip_group_check=True)

            def stage_gate(t, pab):
                """ACT/DVE gating on the a|b psum; logit accumulated into lt."""
                a_sb = smp.tile([128, 128], dt.bfloat16, tag="a")
                t_sb = smp.tile([128, 128], dt.bfloat16, tag="t")
                nc.scalar.activation(a_sb[:], pab[:, 0:128], AF.Relu,
                                     scale=1.0 / W_SCALE)
                nc.scalar.activation(t_sb[:], pab[:, 128:256], AF.Tanh,
                                     scale=0.5 / W_SCALE)
                b = t // BATCH
                j = t % BATCH
                if j == 0:
                    lt_of[b] = ltp.tile([128, BATCH], dt.float32, tag="lt",
                                        name="lt")
                g_sb = smp.tile([128, 128], dt.bfloat16, tag="g")
                gw_sb = smp.tile([128, 128], dt.bfloat16, tag="gw")
                nc.gpsimd.tensor_mul(g_sb[:], a_sb[:], wcbc_sb[:])
                # gw = (t + 1) * g with the free-axis logit reduce fused into
                # the same DVE op via accum_out (stt is DVE-only); kills the
                # separate tensor_scalar AND tensor_reduce of the old chain
                nc.vector.scalar_tensor_tensor(gw_sb[:], t_sb[:], 1.0,
                                               g_sb[:],
                                               op0=mybir.AluOpType.add,
                                               op1=mybir.AluOpType.mult,
                                               accum_out=lt_of[b][:, j:j + 1])

            def stage_pz(b_end):
                """Batched exp -> fp8 pair-weights; returns one closure per
                hD pair so the P/Z matmuls can be spread over the following
                iterations (a single burst of P/Z matmuls at the batch
                boundary head-of-line-blocks the PE queue on the exp/wD
                chain for ~7us)."""
                b = b_end // BATCH
                t0 = b * BATCH
                bs = b_end - t0 + 1
                lt = lt_of.pop(b)
                wtmp = smp.tile([128, BATCH], dt.bfloat16, tag="wtmp")
                nc.scalar.activation(wtmp[:, 0:bs], lt[:, 0:bs], AF.Exp)
                # pair-dim stride must be 16B-aligned for the DR stationary AP
                wD = smp.tile([128, 2, 16], dt.float8e4, tag="wD")
                if bs == BATCH:
                    # wtmp col j -> wD[:, j%2, j//2]
                    nc.vector.tensor_copy(
                        wD[:, :, 0:BATCH // 2].rearrange("p j q -> p q j"),
                        wtmp[:].rearrange("p (q j) -> p q j", j=2))
                else:
                    nc.vector.memset(wD[:], 0.0)
                    for j in range(bs):
                        masked = smp.tile([128, 1], dt.bfloat16, tag="wmask")
                        nc.vector.tensor_mul(masked[:], wtmp[:, j:j + 1], mask_sb[:])
                        nc.vector.tensor_copy(wD[:, j % 2, j // 2:j // 2 + 1], masked[:])
                npairs = (bs + 1) // 2

                def make_pair(p):
                    def emit():
                        gp = t0 // 2 + p
                        hD = hD_of.pop(gp)
                        first = gp == 0
                        last = gp == (T - 1) // 2
                        w_ap = wD[:, :, p:p + 1]
                        nc.tensor.matmul(ppza, w_ap, hD[:, :, 0:512],
                                         start=first, stop=last,
                                         perf_mode=DR, skip_group_check=True)
                        nc.tensor.matmul(ppzb, w_ap, hD[:, :, 512:769],
                                         start=first, stop=last,
                                         perf_mode=DR, skip_group_check=True)
                    return emit
                return [make_pair(p) for p in range(npairs)]

            # main loop + 4 virtual drain iterations that keep the same
            # pipelined emission pattern (no serial tail).  Per iteration i:
            # a|b GEMM + gates for tile i-3 (into/from the psT-bank corner,
            # emitted FIRST so the gate ACTs sit ahead of this tile's h-evac
            # in the ACT queue and ph reuse no longer waits on gates), then
            # the h GEMM of tile i with the transposes of i-1 interleaved.
            # Pipeline lags: transposes of tile i-2 and a|b+gates of tile
            # i-4 are interleaved with the h GEMM of tile i.  The scheduler
            # costs the PE at the mid p-state (2x slower than the real
            # sustained rate), so a 1-tile lag makes the PE arrive at the
            # transposes before their evac has finished; the deeper lag
            # keeps every PE instruction's inputs ready well in advance.
            pz_pairs = []
            for i in range(T + 12):
                if i < T:
                    ph = pshp.tile([128, 768], dt.float32, tag="ph",
                                   name="ph")
                    ph_of[i] = ph
                need_tr = 2 <= i <= T + 1
                need_ab = 4 <= i <= T + 3
                psTs = ([alloc_psT(), alloc_psT()] if need_tr
                        else ([alloc_psT()] if need_ab else []))
                if need_ab:
                    pab = psTs[0][:, 768:1792].bitcast(dt.float32)
                    emit_ab(i - 4, pab)
                    stage_gate(i - 4, pab)
                    ph_of.pop(i - 4, None)
                transp_mm = make_transp(i - 2, psTs) if need_tr else None
                if i < T:
                    stage_h(i, transp_mm)
                elif transp_mm is not None:
                    for c in range(6):
                        transp_mm(c)
                # exp/wD for a finished logit batch, 7 iterations after its
                # last h; the P/Z pair matmuls then drip one per iteration
                # so the PE queue never blocks on the exp chain
                be = i - 7
                if 0 <= be <= T - 1 and (be % BATCH == BATCH - 1 or be == T - 1):
                    pz_pairs.extend(stage_pz(be))
                if pz_pairs:
                    pz_pairs.pop(0)()
                if i >= T + 7 and pz_pairs:
                    # drain faster once the main loop is over
                    pz_pairs.pop(0)()

            out_sb = constp.tile([1, DIM + 1], dt.float32)
            nc.vector.tensor_copy(out_sb[:], ppz[:])
            nc.sync.dma_start(out_d[:], out_sb[:])

    nc.compile()
    return nc


def get_nc():
    global _cached_nc
    if _cached_nc is None:
        _cached_nc = _build_nc()
    return _cached_nc


def make_inputs(x, W_feat, b_feat, W_a, W_b, W_c):
    """Host-side preprocessing: shard + retile x, prepack weights."""
    x = np.asarray(x, dtype=np.float32)
    # fold the bias into x: x' = x + c with c @ W8^T = b_feat where W8 is the
    # DEQUANTIZED fp8 weight actually used on device.  Solving against W8
    # (not the exact W_feat) is essential: c is large (||c||_2 ~ 21), so
    # c @ (W8 - W)^T would otherwise add a systematic offset to every h row
    # (final rel err 0.026 vs 0.0018).
    W8d = ((np.asarray(W_feat, np.float32) * W_SCALE).astype(FP8)
           .astype(np.float64)) / W_SCALE
    c = np.linalg.solve(W8d, np.asarray(b_feat, np.float64)).astype(np.float32)
    xs = x.reshape(N, DIM) + c[None, :]
    NP = T * 128
    xp = np.zeros((N_CORES, NP, DIM), dtype=np.float32)
    xp[:, :NS, :] = xs.reshape(N_CORES, NS, DIM)
    # per tile: block [128 n, 768 d] -> [p, c, n] with d = c*128 + p; the
    # chunk axis c = s*2+j is viewed as [3, 2] for the DoubleRow pairing.
    blocks = xp.reshape(N_CORES, T, 128, 6, 128)             # [core, t, n, c, p]
    xt_host = np.ascontiguousarray(blocks.transpose(0, 1, 4, 3, 2)) \
        .astype(FP8).reshape(N_CORES, T, 128, 3, 2, 128)

    WT = np.asarray(W_feat, np.float32).T * W_SCALE          # [d, e]
    wt_host = np.ascontiguousarray(
        WT.reshape(6, 128, DIM).transpose(1, 0, 2)) \
        .astype(FP8).reshape(128, 3, 2, DIM)

    wab = np.concatenate([np.asarray(W_a, np.float32).T,
                          np.asarray(W_b, np.float32).T], axis=1) * W_SCALE  # [e, 256]
    wab_host = np.ascontiguousarray(
        wab.reshape(3, 2, 128, 2 * D_ATT).transpose(2, 0, 1, 3)).astype(FP8)

    # W_c * 0.5 folds the sigmoid-(1+tanh)/2 halving into the gate constant
    wcbc_host = np.ascontiguousarray(np.tile(
        np.asarray(W_c, np.float32).reshape(1, D_ATT) * 0.5, (128, 1))).astype(BF16)
    mask_host = np.zeros((128, 1), dtype=BF16)
    mask_host[:LAST_VALID] = 1
    ident_host = np.eye(128, dtype=FP8)

    common = dict(wt=wt_host, wab=wab_host, wcbc=wcbc_host,
                  mask=mask_host, ident=ident_host)
    return [dict(xt=np.ascontiguousarray(xt_host[i]), **common)
            for i in range(N_CORES)]


def _ensure_axon_profile_hook():
    """If someone runs kernel() with BASS_TRACE=1 under axon, the spmd runner
    imports antenv.axon_hooks, which this image lacks; shim it from
    trn_agent_boot so tracing degrades gracefully instead of crashing."""
    try:
        import antenv.axon_hooks  # noqa: F401
        return
    except ImportError:
        pass
    try:
        from trn_agent_boot import trn_boot

        hook = trn_boot._ntff_profile_via_ctypes("/opt/axon/libaxon_pjrt.so")
        mod = types.ModuleType("antenv.axon_hooks")
        mod.get_axon_ntff_profile_hook = lambda: hook
        mod.set_axon_ntff_profile_hook = lambda h: None
        sys.modules["antenv.axon_hooks"] = mod
    except Exception:
        pass


def kernel(x, W_feat, b_feat, W_a, W_b, W_c):
    global last_results
    _ensure_axon_profile_hook()
    nc = get_nc()
    in_maps = make_inputs(x, W_feat, b_feat, W_a, W_b, W_c)
    res = run_bass_kernel_spmd(nc, in_maps, core_ids=list(range(N_CORES)))
    last_results = res
    P = np.zeros(DIM, dtype=np.float64)
    Z = 0.0
    for r in res.results:
        o = np.asarray(r["out"], dtype=np.float64).reshape(DIM + 1)
        P += o[:DIM]
        Z += o[DIM]
    return (P / Z).astype(np.float32).reshape(1, DIM)

